# revision 27
# baseline (speedup 1.0000x reference)
"""Bi-directional multi-head cross-attention (GLIP-style) on 8 Trainium2 cores.

Sharding: core c handles (batch = c // 2, v-row half = c % 2) -> 2048 v rows.
 - out_v is fully local to each core (softmax over Nl is within-core).
 - out_l needs reductions over all Nv rows of a batch: each core computes
   unnormalized partial out_l^T = e^T @ val_v and partial colsums c; a 2-rank
   AllReduce per core pair merges them, then both cores finish l2out.

Layouts: activations flow "transposed" (features on partitions) where that
makes every matmul consume natural SBUF layouts with zero runtime transposes
except one PE-transpose of v (and l) at load time.

All matmuls run in float32r (full PE rate, ~1.5e-4 rel err vs fp32).
"""

import contextlib
import sys

sys.path.insert(0, "/opt/trn_rl_repo")

import numpy as np  # noqa: E402

import concourse.bass as bass  # noqa: E402, F401
import concourse.mybir as mybir  # noqa: E402
import concourse.tile as tile  # noqa: E402
from concourse import bacc  # noqa: E402
from concourse.bass_utils import run_bass_kernel_spmd  # noqa: E402
from concourse.masks import make_identity  # noqa: E402

F32 = mybir.dt.float32
F32R = mybir.dt.float32r
EXP = mybir.ActivationFunctionType.Exp
ADD = mybir.AluOpType.add
MULT = mybir.AluOpType.mult
AX = mybir.AxisListType.X

B, NV, NL, DV, DL, E, H, D = 4, 4096, 256, 1024, 768, 1024, 16, 64
NVH = NV // 2  # v rows per core
SCALE = H ** (-0.5)
HP = H // 2  # head pairs
BLK = 512  # q-row block in attention phase
NBLK = NVH // BLK  # 4
QCB = BLK // 128  # q-chunks (128 rows) per block
N_CORES = 8
RG = [[0, 1], [2, 3], [4, 5], [6, 7]]  # core pairs sharing a batch


def _build(flags):
    (f_bq, f_bk, f_bv, f_bl, f_bvo, f_blo, gen_mask) = flags
    nc = bacc.Bacc(
        "TRN2", target_bir_lowering=False, debug=False, num_devices=N_CORES
    )

    v_d = nc.dram_tensor("v", [NVH, DV], F32, kind="ExternalInput").ap()
    l_d = nc.dram_tensor("l", [NL, DL], F32, kind="ExternalInput").ap()
    W_v2q = nc.dram_tensor("W_v2q", [DV, E], F32R, kind="ExternalInput").ap()
    W_l2k = nc.dram_tensor("W_l2k", [DL, E], F32R, kind="ExternalInput").ap()
    W_v2v = nc.dram_tensor("W_v2v", [DV, E], F32R, kind="ExternalInput").ap()
    W_l2v = nc.dram_tensor("W_l2v", [DL, E], F32R, kind="ExternalInput").ap()
    W_v2o = nc.dram_tensor("W_v2out", [E, DV], F32R, kind="ExternalInput").ap()
    W_l2o = nc.dram_tensor("W_l2out", [E, DL], F32R, kind="ExternalInput").ap()
    b_v2q = nc.dram_tensor("b_v2q", [E], F32R, kind="ExternalInput").ap() if f_bq else None
    b_l2k = nc.dram_tensor("b_l2k", [E], F32R, kind="ExternalInput").ap() if f_bk else None
    b_v2v = nc.dram_tensor("b_v2v", [E], F32R, kind="ExternalInput").ap() if f_bv else None
    b_l2v = nc.dram_tensor("b_l2v", [E], F32R, kind="ExternalInput").ap() if f_bl else None
    b_v2o = nc.dram_tensor("b_v2out", [DV], F32R, kind="ExternalInput").ap() if f_bvo else None
    b_l2o = nc.dram_tensor("b_l2out", [DL], F32R, kind="ExternalInput").ap() if f_blo else None
    mask_b = (
        nc.dram_tensor("mask_bias", [2, 128], F32, kind="ExternalInput").ap()
        if gen_mask else None
    )
    mask01 = (
        nc.dram_tensor("mask01", [2, 128], F32, kind="ExternalInput").ap()
        if gen_mask else None
    )
    out_v = nc.dram_tensor("out_v", [NVH, DV], F32, kind="ExternalOutput").ap()
    out_l = nc.dram_tensor("out_l", [NL, DL], F32, kind="ExternalOutput").ap()

    any_bias = any([f_bq, f_bk, f_bv, f_bl, f_bvo, f_blo])

    with tile.TileContext(nc) as tc, contextlib.ExitStack() as stack:
        # ---- whole-kernel resident pool ----
        res = stack.enter_context(tc.tile_pool(name="res", bufs=1))
        ident = res.tile([128, 128], F32, name="ident")
        make_identity(nc, ident[:])
        kT = res.tile([128, 8, NL], F32R, name="kT")  # k^T  [emb, k-rows]
        vall = res.tile([128, 2, E], F32R, name="vall")  # val_l natural [k, emb]
        # out_l^T accum, in per-head 64-partition halves (j = head parity)
        outl_h = [
            res.tile([64, HP, NL], F32, name=f"outl_h{j}") for j in range(2)
        ]
        c_acc = res.tile([128, 2, H, NBLK], F32, name="c_acc")  # colsum partials
        if any_bias:
            ones_f = res.tile([1, 512], F32, name="ones_f")
            nc.vector.memset(ones_f[:], 1.0)
            ones_r = res.tile([1, 512], F32R, name="ones_r")
            nc.vector.tensor_copy(ones_r[:], ones_f[:])
        if gen_mask:
            mb_sb = res.tile([128, 2], F32, name="mb_sb")
            nc.sync.dma_start(mb_sb[:], mask_b.rearrange("kc ki -> ki kc"))
            m01_sb = res.tile([128, 2], F32, name="m01_sb")
            nc.sync.dma_start(m01_sb[:], mask01.rearrange("kc ki -> ki kc"))
            ones128_f = res.tile([128, 1], F32, name="ones128_f")
            nc.vector.memset(ones128_f[:], 1.0)
            ones128 = res.tile([128, 1], F32R, name="ones128")
            nc.vector.tensor_copy(ones128[:], ones128_f[:])

        dram = stack.enter_context(tc.tile_pool(name="dram", bufs=1, space="DRAM"))
        qT_d = dram.tile([128, 8, NVH], F32R, name="qT_d")
        valv_d = dram.tile([128, NVH // 128, E], F32R, name="valv_d")
        ar_in = dram.tile([128, 8 * NL + 2 * H], F32, name="ar_in")
        ar_out = dram.tile([128, 8 * NL + 2 * H], F32, name="ar_out")
        dram_c = dram.tile([H, NL], F32, name="dram_c")

        def bcast_rows_to_div(div_dst, rows_src, hp_count, n):
            # div_dst [128, hp_count, n] <- rows_src [2*hp_count, n] (DRAM),
            # partition p of slice hp reads row 2*hp + (p >= 64).
            src = rows_src.rearrange("(hp j) q -> j hp q", j=2)
            for j in range(2):
                nc.sync.dma_start(
                    div_dst[64 * j:64 * (j + 1)],
                    src[j, None, :, :].to_broadcast([64, hp_count, n]),
                )

        def bias_mm_t(ps, b_sb, mslice, nsize):
            # transposed-layout output: out[emb, rows] += b[emb]
            nc.tensor.matmul(
                ps, b_sb[0:1, mslice], ones_r[0:1, 0:nsize], start=False, stop=True
            )

        def bias_mm_n(ps, b_sb, nslice, msize):
            # natural-layout output: out[rows, emb] += b[emb]
            nc.tensor.matmul(
                ps, ones_r[0:1, 0:msize], b_sb[0:1, nslice], start=False, stop=True
            )

        # ================= phase 0: l-side projections =================
        with tc.tile_pool(name="ph0", bufs=1) as p0, \
             tc.tile_pool(name="ps0", bufs=1, space="PSUM") as ps0:
            l_nat = p0.tile([128, 2, DL], F32, name="l_nat")
            nc.sync.dma_start(l_nat[:], l_d.rearrange("(ko ki) f -> ki ko f", ki=128))
            lT = p0.tile([128, 6, NL], F32R, name="lT")
            for fc in range(6):
                for lc in range(2):
                    pt = ps0.tile([128, 128], F32, tag="tr", bufs=2, name="pt")
                    nc.tensor.transpose(
                        pt[:], l_nat[:, lc, fc * 128:(fc + 1) * 128], ident[:]
                    )
                    nc.vector.tensor_copy(lT[:, fc, lc * 128:(lc + 1) * 128], pt[:])
            Wk_sb = p0.tile([128, 6, E], F32R, name="Wk_sb")
            nc.sync.dma_start(Wk_sb[:], W_l2k.rearrange("(kc ki) m -> ki kc m", ki=128))
            Wlv_sb = p0.tile([128, 6, E], F32R, name="Wlv_sb")
            nc.sync.dma_start(Wlv_sb[:], W_l2v.rearrange("(kc ki) m -> ki kc m", ki=128))
            bk_sb = None
            if f_bk:
                bk_sb = p0.tile([1, E], F32R, name="bk_sb")
                nc.sync.dma_start(bk_sb[:], b_l2k[None, :])
            blv_sb = None
            if f_bl:
                blv_sb = p0.tile([1, E], F32R, name="blv_sb")
                nc.sync.dma_start(blv_sb[:], b_l2v[None, :])
            for mc in range(8):
                pk = ps0.tile([128, NL], F32, tag="pk", bufs=2, name="pk")
                for kc in range(6):
                    nc.tensor.matmul(
                        pk[:],
                        Wk_sb[:, kc, mc * 128:(mc + 1) * 128],
                        lT[:, kc, :],
                        start=(kc == 0),
                        stop=(kc == 5 and not f_bk),
                    )
                if f_bk:
                    bias_mm_t(pk[:], bk_sb, slice(mc * 128, (mc + 1) * 128), NL)
                nc.vector.tensor_copy(kT[:, mc, :], pk[:])
            for mc2 in range(2):
                for nch in range(2):
                    pv = ps0.tile([128, 512], F32, tag="pv", bufs=2, name="pv")
                    for kc in range(6):
                        nc.tensor.matmul(
                            pv[:],
                            lT[:, kc, mc2 * 128:(mc2 + 1) * 128],
                            Wlv_sb[:, kc, nch * 512:(nch + 1) * 512],
                            start=(kc == 0),
                            stop=(kc == 5 and not f_bl),
                        )
                    if f_bl:
                        bias_mm_n(pv[:], blv_sb, slice(nch * 512, (nch + 1) * 512), 128)
                    nc.vector.tensor_copy(
                        vall[:, mc2, nch * 512:(nch + 1) * 512], pv[:]
                    )

        # ================= phase 1: v-side projections =================
        with tc.tile_pool(name="ph1", bufs=1) as p1, \
             tc.tile_pool(name="ps1", bufs=1, space="PSUM") as ps1:
            Wq_sb = p1.tile([128, 8, E], F32R, name="Wq_sb")
            nc.sync.dma_start(Wq_sb[:], W_v2q.rearrange("(kc ki) m -> ki kc m", ki=128))
            Wvv_sb = p1.tile([128, 8, E], F32R, name="Wvv_sb")
            nc.sync.dma_start(Wvv_sb[:], W_v2v.rearrange("(kc ki) m -> ki kc m", ki=128))
            bq_sb = None
            if f_bq:
                bq_sb = p1.tile([1, E], F32R, name="bq_sb")
                nc.sync.dma_start(bq_sb[:], b_v2q[None, :])
            bvv_sb = None
            if f_bv:
                bvv_sb = p1.tile([1, E], F32R, name="bvv_sb")
                nc.sync.dma_start(bvv_sb[:], b_v2v[None, :])
            for blk in range(NBLK):
                v_nat = p1.tile([128, QCB, DV], F32, tag="vnat", bufs=2, name="v_nat")
                nc.sync.dma_start(
                    v_nat[:],
                    v_d[blk * BLK:(blk + 1) * BLK, :].rearrange(
                        "(qo qi) f -> qi qo f", qi=128
                    ),
                )
                vT = p1.tile([128, 8, BLK], F32R, tag="vT", bufs=2, name="vT")
                for fc in range(8):
                    for qc in range(QCB):
                        pt = ps1.tile([128, 128], F32, tag="tr", bufs=3, name="pt1")
                        nc.tensor.transpose(
                            pt[:], v_nat[:, qc, fc * 128:(fc + 1) * 128], ident[:]
                        )
                        nc.vector.tensor_copy(
                            vT[:, fc, qc * 128:(qc + 1) * 128], pt[:]
                        )
                for mc in range(8):
                    pq = ps1.tile([128, BLK], F32, tag="pq", bufs=2, name="pq")
                    for kc in range(8):
                        nc.tensor.matmul(
                            pq[:],
                            Wq_sb[:, kc, mc * 128:(mc + 1) * 128],
                            vT[:, kc, :],
                            start=(kc == 0),
                            stop=(kc == 7 and not f_bq),
                        )
                    if f_bq:
                        bias_mm_t(pq[:], bq_sb, slice(mc * 128, (mc + 1) * 128), BLK)
                    qt_ev = p1.tile([128, BLK], F32R, tag="qtev", bufs=3, name="qt_ev")
                    nc.vector.tensor_copy(qt_ev[:], pq[:])
                    nc.sync.dma_start(
                        qT_d[:, mc, blk * BLK:(blk + 1) * BLK], qt_ev[:]
                    )
                for qc in range(QCB):
                    for nch in range(2):
                        pvv = ps1.tile([128, 512], F32, tag="pvv", bufs=2, name="pvv")
                        for kc in range(8):
                            nc.tensor.matmul(
                                pvv[:],
                                vT[:, kc, qc * 128:(qc + 1) * 128],
                                Wvv_sb[:, kc, nch * 512:(nch + 1) * 512],
                                start=(kc == 0),
                                stop=(kc == 7 and not f_bv),
                            )
                        if f_bv:
                            bias_mm_n(
                                pvv[:], bvv_sb, slice(nch * 512, (nch + 1) * 512), 128
                            )
                        vv_ev = p1.tile(
                            [128, 512], F32R, tag="vvev", bufs=3, name="vv_ev"
                        )
                        nc.vector.tensor_copy(vv_ev[:], pvv[:])
                        nc.sync.dma_start(
                            valv_d[:, blk * QCB + qc, nch * 512:(nch + 1) * 512],
                            vv_ev[:],
                        )

        # ================= phase 2: attention =================
        with tc.tile_pool(name="ph2", bufs=1) as p2, \
             tc.tile_pool(name="ps2w", bufs=1, space="PSUM") as psW, \
             tc.tile_pool(name="ps2a", bufs=1, space="PSUM") as psA:
            Wvo_sb = p2.tile([128, 8, DV], F32R, name="Wvo_sb")
            nc.sync.dma_start(Wvo_sb[:], W_v2o.rearrange("(kc ki) m -> ki kc m", ki=128))
            bvo_sb = None
            if f_bvo:
                bvo_sb = p2.tile([1, DV], F32R, name="bvo_sb")
                nc.sync.dma_start(bvo_sb[:], b_v2o[None, :])
            for blk in range(NBLK):
                qT_b = p2.tile([128, 8, BLK], F32R, tag="qTb", bufs=1, name="qT_b")
                nc.sync.dma_start(qT_b[:], qT_d[:, :, blk * BLK:(blk + 1) * BLK])
                vv_b = p2.tile([128, QCB, E], F32R, tag="vvb", bufs=2, name="vv_b")
                nc.sync.dma_start(vv_b[:], valv_d[:, blk * QCB:(blk + 1) * QCB, :])
                r_acc = p2.tile([128, QCB, H], F32, tag="racc", bufs=2, name="r_acc")

                # ---- pass A: scores [q,k], e, out_l accumulation ----
                for hp in range(HP):
                    pol = [
                        psA.tile([64, NL], F32, tag="acc", bufs=4, name=f"pol{j}")
                        for j in range(2)
                    ]
                    for qc in range(QCB):
                        for j in range(2):
                            h = 2 * hp + j
                            ps_s = psW.tile([128, NL], F32, tag="wk", bufs=4, name="ps_s")
                            nc.tensor.matmul(
                                ps_s[:],
                                qT_b[64 * j:64 * (j + 1), hp, qc * 128:(qc + 1) * 128],
                                kT[64 * j:64 * (j + 1), hp, :],
                                start=True,
                                stop=True,
                                tile_position=(64 * j, 0),
                            )
                            e_t = p2.tile([128, NL], F32R, tag="e", bufs=4, name="e_t")
                            nc.scalar.activation(
                                e_t[:], ps_s[:], EXP, scale=SCALE,
                                accum_out=r_acc[:, qc, h:h + 1],
                            )
                            nc.tensor.matmul(
                                pol[j][:],
                                vv_b[:, qc, 64 * h:64 * h + 64],
                                e_t[:],
                                start=(qc == 0),
                                stop=(qc == QCB - 1),
                            )
                    for j in range(2):
                        if blk == 0:
                            nc.vector.tensor_copy(outl_h[j][:, hp, :], pol[j][:])
                        else:
                            nc.vector.tensor_tensor(
                                outl_h[j][:, hp, :], outl_h[j][:, hp, :],
                                pol[j][:], ADD,
                            )

                # ---- r -> reciprocal -> row layout -> divisor tiles ----
                if not gen_mask:
                    rrows = p2.tile([16, BLK], F32, tag="rrows", bufs=2, name="rrows")
                    for qc in range(QCB):
                        r_rec = p2.tile([128, H], F32, tag="rrec", bufs=2, name="r_rec")
                        nc.vector.reciprocal(r_rec[:], r_acc[:, qc, :])
                        pr = psW.tile([16, 128], F32, tag="wk", bufs=4, name="pr")
                        nc.tensor.transpose(pr[:], r_rec[:], ident[:])
                        nc.vector.tensor_copy(
                            rrows[:, qc * 128:(qc + 1) * 128], pr[:]
                        )
                    dram_r = dram.tile([H, BLK], F32, tag="dram_r", bufs=2, name="dram_r")
                    nc.sync.dma_start(dram_r[:], rrows[:])

                # ---- pass B: scores^T, e^T, out_v^T, v2out ----
                outv_h = [
                    p2.tile([64, HP, BLK], F32R, tag=f"outvh{j}", bufs=1,
                            name=f"outv_h{j}")
                    for j in range(2)
                ]
                div64 = None
                for hp in range(HP):
                    if not gen_mask and hp % 4 == 0:
                        # 1/r rows for heads [8*(hp//4), 8*(hp//4)+8), bcast
                        # over 64 partitions; two half-loads to save SBUF
                        div64 = p2.tile(
                            [64, 8, BLK], F32, tag="divall", bufs=1, name="div64"
                        )
                        nc.sync.dma_start(
                            div64[:],
                            dram_r[None, 8 * (hp // 4):8 * (hp // 4) + 8, :]
                            .to_broadcast([64, 8, BLK]),
                        )
                    pov = [
                        psA.tile([64, BLK], F32, tag="acc", bufs=4, name=f"pov{j}")
                        for j in range(2)
                    ]
                    eT_pair = []
                    for j in range(2):
                        h = 2 * hp + j
                        for kc in range(2):
                            ps_t = psW.tile([128, BLK], F32, tag="wk", bufs=4, name="ps_t")
                            nc.tensor.matmul(
                                ps_t[:],
                                kT[64 * j:64 * (j + 1), hp, kc * 128:(kc + 1) * 128],
                                qT_b[64 * j:64 * (j + 1), hp, :],
                                start=True,
                                stop=True,
                                tile_position=(64 * j, 0),
                            )
                            eT_t = p2.tile([128, BLK], F32R, tag="eT", bufs=4, name="eT_t")
                            nc.scalar.activation(
                                eT_t[:], ps_t[:], EXP, scale=SCALE,
                                accum_out=c_acc[:, kc, h, blk:blk + 1],
                            )
                            if gen_mask:
                                nc.vector.tensor_scalar_mul(
                                    eT_t[:], eT_t[:], m01_sb[:, kc:kc + 1]
                                )
                            nc.tensor.matmul(
                                pov[j][:],
                                vall[:, kc, 64 * h:64 * h + 64],
                                eT_t[:],
                                start=(kc == 0),
                                stop=(kc == 1),
                            )
                            eT_pair.append(eT_t)
                    if gen_mask:
                        # masked row-sums via ones-matmul -> recip -> DRAM rows
                        dram_rg = dram.tile(
                            [2, BLK], F32, tag="dram_rg", bufs=3, name="dram_rg"
                        )
                        for j in range(2):
                            prr = psW.tile([1, BLK], F32, tag="wk", bufs=4, name="prr")
                            for kc in range(2):
                                nc.tensor.matmul(
                                    prr[:],
                                    ones128[:],
                                    eT_pair[2 * j + kc][:],
                                    start=(kc == 0),
                                    stop=(kc == 1),
                                )
                            rr_sb = p2.tile([1, BLK], F32, tag="rrsb", bufs=3, name="rr_sb")
                            nc.vector.reciprocal(rr_sb[:], prr[:])
                            nc.sync.dma_start(dram_rg[j:j + 1, :], rr_sb[:])
                        div_g = p2.tile([64, 2, BLK], F32, tag="divhp", bufs=3, name="div_g")
                        nc.sync.dma_start(
                            div_g[:], dram_rg[None, :, :].to_broadcast([64, 2, BLK])
                        )
                        divs = [div_g[:, 0, :], div_g[:, 1, :]]
                    else:
                        divs = [
                            div64[:, (2 * hp) % 8, :],
                            div64[:, (2 * hp + 1) % 8, :],
                        ]
                    for j in range(2):
                        nc.vector.tensor_tensor(
                            outv_h[j][:, hp, :], pov[j][:], divs[j], MULT
                        )
                # repack the two 64-partition halves into [128, 8, BLK]
                outv_sb = p2.tile([128, 8, BLK], F32R, tag="outv", bufs=1, name="outv_sb")
                for j in range(2):
                    nc.sync.dma_start(
                        outv_sb[64 * j:64 * (j + 1), :, :], outv_h[j][:]
                    )

                for qc in range(QCB):
                    for nch in range(2):
                        pf = psW.tile([128, 512], F32, tag="wk", bufs=4, name="pf")
                        for kc in range(8):
                            nc.tensor.matmul(
                                pf[:],
                                outv_sb[:, kc, qc * 128:(qc + 1) * 128],
                                Wvo_sb[:, kc, nch * 512:(nch + 1) * 512],
                                start=(kc == 0),
                                stop=(kc == 7 and not f_bvo),
                            )
                        if f_bvo:
                            bias_mm_n(
                                pf[:], bvo_sb, slice(nch * 512, (nch + 1) * 512), 128
                            )
                        fv = p2.tile([128, 512], F32, tag="fv", bufs=3, name="fv")
                        nc.vector.tensor_copy(fv[:], pf[:])
                        nc.sync.dma_start(
                            out_v[
                                blk * BLK + qc * 128:blk * BLK + (qc + 1) * 128,
                                nch * 512:(nch + 1) * 512,
                            ],
                            fv[:],
                        )

        # ================= phase 3: out_l merge + projection =================
        with tc.tile_pool(name="ph3", bufs=1) as p3, \
             tc.tile_pool(name="ps3", bufs=1, space="PSUM") as ps3:
            c_sum = p3.tile([128, 2, H], F32, name="c_sum")
            nc.vector.tensor_reduce(c_sum[:], c_acc[:], axis=AX, op=ADD)
            outl128 = p3.tile([128, HP, NL], F32, name="outl128")
            for j in range(2):
                nc.sync.dma_start(
                    outl128[64 * j:64 * (j + 1), :, :], outl_h[j][:]
                )
            nc.sync.dma_start(
                ar_in[:, 0:8 * NL], outl128.rearrange("p a b -> p (a b)")
            )
            nc.sync.dma_start(
                ar_in[:, 8 * NL:8 * NL + 2 * H], c_sum.rearrange("p a b -> p (a b)")
            )
            nc.gpsimd.collective_compute(
                "AllReduce",
                ADD,
                replica_groups=RG,
                ins=[ar_in.opt()],
                outs=[ar_out.opt()],
            )
            outl_sum = p3.tile([128, 8, NL], F32, name="outl_sum")
            nc.sync.dma_start(
                outl_sum.rearrange("p a b -> p (a b)"), ar_out[:, 0:8 * NL]
            )
            c_fin = p3.tile([128, 2, H], F32, name="c_fin")
            nc.sync.dma_start(
                c_fin.rearrange("p a b -> p (a b)"), ar_out[:, 8 * NL:8 * NL + 2 * H]
            )
            c_rec = p3.tile([128, 2, H], F32, name="c_rec")
            nc.vector.reciprocal(c_rec[:], c_fin[:])
            crows = p3.tile([16, NL], F32, name="crows")
            for kc in range(2):
                pc = ps3.tile([16, 128], F32, tag="wk3", bufs=2, name="pc")
                nc.tensor.transpose(pc[:], c_rec[:, kc, :], ident[:])
                nc.vector.tensor_copy(crows[:, kc * 128:(kc + 1) * 128], pc[:])
            nc.sync.dma_start(dram_c[:], crows[:])
            divl_all = p3.tile([128, HP, NL], F32, name="divl_all")
            bcast_rows_to_div(divl_all, dram_c, HP, NL)
            outl_n = p3.tile([128, 8, NL], F32R, name="outl_n")
            for hp in range(HP):
                nc.vector.tensor_tensor(
                    outl_n[:, hp, :], outl_sum[:, hp, :], divl_all[:, hp, :], MULT
                )
            Wlo_sb = p3.tile([128, 8, DL], F32R, name="Wlo_sb")
            nc.sync.dma_start(Wlo_sb[:], W_l2o.rearrange("(kc ki) m -> ki kc m", ki=128))
            blo_sb = None
            if f_blo:
                blo_sb = p3.tile([1, DL], F32R, name="blo_sb")
                nc.sync.dma_start(blo_sb[:], b_l2o[None, :])
            for mc2 in range(2):
                for n0, nw in ((0, 512), (512, 256)):
                    pl = ps3.tile([128, 512], F32, tag="wk3", bufs=2, name="pl")
                    for kc in range(8):
                        nc.tensor.matmul(
                            pl[:, 0:nw],
                            outl_n[:, kc, mc2 * 128:(mc2 + 1) * 128],
                            Wlo_sb[:, kc, n0:n0 + nw],
                            start=(kc == 0),
                            stop=(kc == 7 and not f_blo),
                        )
                    if f_blo:
                        bias_mm_n(pl[:, 0:nw], blo_sb, slice(n0, n0 + nw), 128)
                    fl = p3.tile([128, 512], F32, tag="fl", bufs=2, name="fl")
                    nc.vector.tensor_copy(fl[:, 0:nw], pl[:, 0:nw])
                    nc.sync.dma_start(
                        out_l[mc2 * 128:(mc2 + 1) * 128, n0:n0 + nw], fl[:, 0:nw]
                    )

    nc.compile()
    return nc


_cache = {}
_PROFILE = False
LAST_EXEC_NS = None


def _get_nc(flags):
    if flags not in _cache:
        _cache[flags] = _build(flags)
    return _cache[flags]


def kernel(**inputs):
    v = np.asarray(inputs["v"], dtype=np.float32)
    l = np.asarray(inputs["l"], dtype=np.float32)
    mask = np.asarray(inputs["attention_mask_l"])
    ws = {
        k: np.ascontiguousarray(np.asarray(inputs[k], dtype=np.float32))
        for k in ["W_v2q", "W_l2k", "W_v2v", "W_l2v", "W_v2out", "W_l2out"]
    }
    bs = {
        k: np.asarray(inputs[k], dtype=np.float32)
        for k in ["b_v2q", "b_l2k", "b_v2v", "b_l2v", "b_v2out", "b_l2out"]
    }
    flags_b = tuple(
        bool(np.any(bs[k]))
        for k in ["b_v2q", "b_l2k", "b_v2v", "b_l2v", "b_v2out", "b_l2out"]
    )
    gen_mask = bool(np.any(mask == 0))
    flags = flags_b + (gen_mask,)
    nc = _get_nc(flags)

    in_maps = []
    for c in range(N_CORES):
        b, half = c // 2, c % 2
        m = {
            "v": np.ascontiguousarray(v[b, half * NVH:(half + 1) * NVH]),
            "l": np.ascontiguousarray(l[b]),
        }
        m.update(ws)
        for k, f in zip(["b_v2q", "b_l2k", "b_v2v", "b_l2v", "b_v2out", "b_l2out"],
                        flags_b):
            if f:
                m[k] = bs[k]
        if gen_mask:
            mk = mask[b]
            m["mask_bias"] = np.where(mk == 0, -1e9, 0.0).astype(np.float32).reshape(2, 128)
            m["mask01"] = (mk != 0).astype(np.float32).reshape(2, 128)
        in_maps.append(m)

    global LAST_EXEC_NS
    res = run_bass_kernel_spmd(
        nc, in_maps, list(range(N_CORES)), trace=_PROFILE
    )
    LAST_EXEC_NS = res.exec_time_ns
    out_v = np.empty((B, NV, DV), dtype=np.float32)
    out_l = np.empty((B, NL, DL), dtype=np.float32)
    for c in range(N_CORES):
        b, half = c // 2, c % 2
        out_v[b, half * NVH:(half + 1) * NVH] = res.results[c]["out_v"]
        if half == 0:
            out_l[b] = res.results[c]["out_l"]
    return out_v, out_l


# revision 33
# speedup vs baseline: 1.0317x; 1.0317x over previous
"""Bi-directional multi-head cross-attention (GLIP-style) on 8 Trainium2 cores.

Sharding: core c handles (batch = c // 2, v-row half = c % 2) -> 2048 v rows.
 - out_v is fully local to each core (softmax over Nl is within-core).
 - out_l needs reductions over all Nv rows of a batch: each core computes
   unnormalized partial out_l^T = e^T @ val_v and partial colsums c; a 2-rank
   AllReduce per core pair merges them, then both cores finish l2out.

Layouts: activations flow "transposed" (features on partitions) where that
makes every matmul consume natural SBUF layouts with zero runtime transposes
except one PE-transpose of v (and l) at load time.

All matmuls run in float32r (full PE rate, ~1.5e-4 rel err vs fp32).
"""

import contextlib
import sys

sys.path.insert(0, "/opt/trn_rl_repo")

import numpy as np  # noqa: E402

import concourse.bass as bass  # noqa: E402, F401
import concourse.mybir as mybir  # noqa: E402
import concourse.tile as tile  # noqa: E402
from concourse import bacc  # noqa: E402
from concourse.bass_utils import run_bass_kernel_spmd  # noqa: E402
from concourse.masks import make_identity  # noqa: E402

F32 = mybir.dt.float32
F32R = mybir.dt.float32r
EXP = mybir.ActivationFunctionType.Exp
ADD = mybir.AluOpType.add
MULT = mybir.AluOpType.mult
AX = mybir.AxisListType.X

B, NV, NL, DV, DL, E, H, D = 4, 4096, 256, 1024, 768, 1024, 16, 64
NVH = NV // 2  # v rows per core
SCALE = H ** (-0.5)
HP = H // 2  # head pairs
BLK = 512  # q-row block in attention phase
NBLK = NVH // BLK  # 4
QCB = BLK // 128  # q-chunks (128 rows) per block
N_CORES = 8
RG = [[0, 1], [2, 3], [4, 5], [6, 7]]  # core pairs sharing a batch


def _build(flags):
    (f_bq, f_bk, f_bv, f_bl, f_bvo, f_blo, gen_mask) = flags
    nc = bacc.Bacc(
        "TRN2", target_bir_lowering=False, debug=False, num_devices=N_CORES
    )

    v_d = nc.dram_tensor("v", [NVH, DV], F32, kind="ExternalInput").ap()
    l_d = nc.dram_tensor("l", [NL, DL], F32, kind="ExternalInput").ap()
    W_v2q = nc.dram_tensor("W_v2q", [DV, E], F32R, kind="ExternalInput").ap()
    W_l2k = nc.dram_tensor("W_l2k", [DL, E], F32R, kind="ExternalInput").ap()
    W_v2v = nc.dram_tensor("W_v2v", [DV, E], F32R, kind="ExternalInput").ap()
    W_l2v = nc.dram_tensor("W_l2v", [DL, E], F32R, kind="ExternalInput").ap()
    W_v2o = nc.dram_tensor("W_v2out", [E, DV], F32R, kind="ExternalInput").ap()
    W_l2o = nc.dram_tensor("W_l2out", [E, DL], F32R, kind="ExternalInput").ap()
    b_v2q = nc.dram_tensor("b_v2q", [E], F32R, kind="ExternalInput").ap() if f_bq else None
    b_l2k = nc.dram_tensor("b_l2k", [E], F32R, kind="ExternalInput").ap() if f_bk else None
    b_v2v = nc.dram_tensor("b_v2v", [E], F32R, kind="ExternalInput").ap() if f_bv else None
    b_l2v = nc.dram_tensor("b_l2v", [E], F32R, kind="ExternalInput").ap() if f_bl else None
    b_v2o = nc.dram_tensor("b_v2out", [DV], F32R, kind="ExternalInput").ap() if f_bvo else None
    b_l2o = nc.dram_tensor("b_l2out", [DL], F32R, kind="ExternalInput").ap() if f_blo else None
    mask_b = (
        nc.dram_tensor("mask_bias", [2, 128], F32, kind="ExternalInput").ap()
        if gen_mask else None
    )
    mask01 = (
        nc.dram_tensor("mask01", [2, 128], F32, kind="ExternalInput").ap()
        if gen_mask else None
    )
    out_v = nc.dram_tensor("out_v", [NVH, DV], F32, kind="ExternalOutput").ap()
    out_l = nc.dram_tensor("out_l", [NL, DL], F32, kind="ExternalOutput").ap()

    any_bias = any([f_bq, f_bk, f_bv, f_bl, f_bvo, f_blo])

    with tile.TileContext(nc) as tc, contextlib.ExitStack() as stack:
        # ---- whole-kernel resident pool ----
        res = stack.enter_context(tc.tile_pool(name="res", bufs=1))
        ident = res.tile([128, 128], F32, name="ident")
        make_identity(nc, ident[:])
        kT = res.tile([128, 8, NL], F32R, name="kT")  # k^T  [emb, k-rows]
        vall = res.tile([128, 2, E], F32R, name="vall")  # val_l natural [k, emb]
        # out_l^T accum, in per-head 64-partition halves (j = head parity)
        outl_h = [
            res.tile([64, HP, NL], F32, name=f"outl_h{j}") for j in range(2)
        ]
        c_acc = res.tile([128, 2, H, NBLK], F32, name="c_acc")  # colsum partials
        if any_bias:
            ones_f = res.tile([1, 512], F32, name="ones_f")
            nc.vector.memset(ones_f[:], 1.0)
            ones_r = res.tile([1, 512], F32R, name="ones_r")
            nc.vector.tensor_copy(ones_r[:], ones_f[:])
        if gen_mask:
            mb_sb = res.tile([128, 2], F32, name="mb_sb")
            nc.sync.dma_start(mb_sb[:], mask_b.rearrange("kc ki -> ki kc"))
            m01_sb = res.tile([128, 2], F32, name="m01_sb")
            nc.sync.dma_start(m01_sb[:], mask01.rearrange("kc ki -> ki kc"))
            ones128_f = res.tile([128, 1], F32, name="ones128_f")
            nc.vector.memset(ones128_f[:], 1.0)
            ones128 = res.tile([128, 1], F32R, name="ones128")
            nc.vector.tensor_copy(ones128[:], ones128_f[:])

        # v2out weights persist through phase 2; load starts immediately
        wvo = stack.enter_context(tc.tile_pool(name="wvo", bufs=1))
        Wvo_sb = wvo.tile([128, 8, DV], F32R, name="Wvo_sb")
        nc.sync.dma_start(Wvo_sb[:], W_v2o.rearrange("(kc ki) m -> ki kc m", ki=128))
        # v2q/v2v weights: freed after phase 1 (nested pool, closed below)
        wres1 = tc.tile_pool(name="wres1", bufs=1)
        w1 = wres1.__enter__()
        Wq_sb = w1.tile([128, 8, E], F32R, name="Wq_sb")
        nc.sync.dma_start(Wq_sb[:], W_v2q.rearrange("(kc ki) m -> ki kc m", ki=128))
        Wvv_sb = w1.tile([128, 8, E], F32R, name="Wvv_sb")
        nc.sync.dma_start(Wvv_sb[:], W_v2v.rearrange("(kc ki) m -> ki kc m", ki=128))

        dram = stack.enter_context(tc.tile_pool(name="dram", bufs=1, space="DRAM"))
        qT_d = dram.tile([128, 8, NVH], F32R, name="qT_d")
        valv_d = dram.tile([128, NVH // 128, E], F32R, name="valv_d")
        ar_in = dram.tile([128, 8 * NL + 2 * H], F32, name="ar_in")
        ar_out = dram.tile([128, 8 * NL + 2 * H], F32, name="ar_out")
        dram_c = dram.tile([H, NL], F32, name="dram_c")

        def bcast_rows_to_div(div_dst, rows_src, hp_count, n):
            # div_dst [128, hp_count, n] <- rows_src [2*hp_count, n] (DRAM),
            # partition p of slice hp reads row 2*hp + (p >= 64).
            src = rows_src.rearrange("(hp j) q -> j hp q", j=2)
            for j in range(2):
                nc.sync.dma_start(
                    div_dst[64 * j:64 * (j + 1)],
                    src[j, None, :, :].to_broadcast([64, hp_count, n]),
                )

        def bias_mm_t(ps, b_sb, mslice, nsize):
            # transposed-layout output: out[emb, rows] += b[emb]
            nc.tensor.matmul(
                ps, b_sb[0:1, mslice], ones_r[0:1, 0:nsize], start=False, stop=True
            )

        def bias_mm_n(ps, b_sb, nslice, msize):
            # natural-layout output: out[rows, emb] += b[emb]
            nc.tensor.matmul(
                ps, ones_r[0:1, 0:msize], b_sb[0:1, nslice], start=False, stop=True
            )

        # ================= phase 0: l-side projections =================
        with tc.tile_pool(name="ph0", bufs=1) as p0, \
             tc.tile_pool(name="ps0", bufs=1, space="PSUM") as ps0:
            l_nat = p0.tile([128, 2, DL], F32, name="l_nat")
            nc.sync.dma_start(l_nat[:], l_d.rearrange("(ko ki) f -> ki ko f", ki=128))
            lT = p0.tile([128, 6, NL], F32R, name="lT")
            for fc in range(6):
                for lc in range(2):
                    pt = ps0.tile([128, 128], F32, tag="tr", bufs=2, name="pt")
                    nc.tensor.transpose(
                        pt[:], l_nat[:, lc, fc * 128:(fc + 1) * 128], ident[:]
                    )
                    nc.vector.tensor_copy(lT[:, fc, lc * 128:(lc + 1) * 128], pt[:])
            Wk_sb = p0.tile([128, 6, E], F32R, name="Wk_sb")
            nc.sync.dma_start(Wk_sb[:], W_l2k.rearrange("(kc ki) m -> ki kc m", ki=128))
            Wlv_sb = p0.tile([128, 6, E], F32R, name="Wlv_sb")
            nc.sync.dma_start(Wlv_sb[:], W_l2v.rearrange("(kc ki) m -> ki kc m", ki=128))
            bk_sb = None
            if f_bk:
                bk_sb = p0.tile([1, E], F32R, name="bk_sb")
                nc.sync.dma_start(bk_sb[:], b_l2k[None, :])
            blv_sb = None
            if f_bl:
                blv_sb = p0.tile([1, E], F32R, name="blv_sb")
                nc.sync.dma_start(blv_sb[:], b_l2v[None, :])
            for mc in range(8):
                pk = ps0.tile([128, NL], F32, tag="pk", bufs=2, name="pk")
                for kc in range(6):
                    nc.tensor.matmul(
                        pk[:],
                        Wk_sb[:, kc, mc * 128:(mc + 1) * 128],
                        lT[:, kc, :],
                        start=(kc == 0),
                        stop=(kc == 5 and not f_bk),
                    )
                if f_bk:
                    bias_mm_t(pk[:], bk_sb, slice(mc * 128, (mc + 1) * 128), NL)
                nc.vector.tensor_copy(kT[:, mc, :], pk[:])
            for mc2 in range(2):
                for nch in range(2):
                    pv = ps0.tile([128, 512], F32, tag="pv", bufs=2, name="pv")
                    for kc in range(6):
                        nc.tensor.matmul(
                            pv[:],
                            lT[:, kc, mc2 * 128:(mc2 + 1) * 128],
                            Wlv_sb[:, kc, nch * 512:(nch + 1) * 512],
                            start=(kc == 0),
                            stop=(kc == 5 and not f_bl),
                        )
                    if f_bl:
                        bias_mm_n(pv[:], blv_sb, slice(nch * 512, (nch + 1) * 512), 128)
                    nc.vector.tensor_copy(
                        vall[:, mc2, nch * 512:(nch + 1) * 512], pv[:]
                    )

        # ================= phase 1: v-side projections =================
        with tc.tile_pool(name="ph1", bufs=1) as p1, \
             tc.tile_pool(name="ps1", bufs=1, space="PSUM") as ps1:
            bq_sb = None
            if f_bq:
                bq_sb = p1.tile([1, E], F32R, name="bq_sb")
                nc.sync.dma_start(bq_sb[:], b_v2q[None, :])
            bvv_sb = None
            if f_bv:
                bvv_sb = p1.tile([1, E], F32R, name="bvv_sb")
                nc.sync.dma_start(bvv_sb[:], b_v2v[None, :])
            for blk in range(NBLK):
                v_nat = p1.tile([128, QCB, DV], F32, tag="vnat", bufs=2, name="v_nat")
                nc.sync.dma_start(
                    v_nat[:],
                    v_d[blk * BLK:(blk + 1) * BLK, :].rearrange(
                        "(qo qi) f -> qi qo f", qi=128
                    ),
                )
                vT = p1.tile([128, 8, BLK], F32R, tag="vT", bufs=2, name="vT")
                for fc in range(8):
                    for qc in range(QCB):
                        pt = ps1.tile([128, 128], F32, tag="tr", bufs=3, name="pt1")
                        nc.tensor.transpose(
                            pt[:], v_nat[:, qc, fc * 128:(fc + 1) * 128], ident[:]
                        )
                        nc.vector.tensor_copy(
                            vT[:, fc, qc * 128:(qc + 1) * 128], pt[:]
                        )
                for mc in range(8):
                    pq = ps1.tile([128, BLK], F32, tag="pq", bufs=2, name="pq")
                    for kc in range(8):
                        nc.tensor.matmul(
                            pq[:],
                            Wq_sb[:, kc, mc * 128:(mc + 1) * 128],
                            vT[:, kc, :],
                            start=(kc == 0),
                            stop=(kc == 7 and not f_bq),
                        )
                    if f_bq:
                        bias_mm_t(pq[:], bq_sb, slice(mc * 128, (mc + 1) * 128), BLK)
                    qt_ev = p1.tile([128, BLK], F32R, tag="qtev", bufs=3, name="qt_ev")
                    nc.vector.tensor_copy(qt_ev[:], pq[:])
                    nc.sync.dma_start(
                        qT_d[:, mc, blk * BLK:(blk + 1) * BLK], qt_ev[:]
                    )
                for qc in range(QCB):
                    for nch in range(2):
                        pvv = ps1.tile([128, 512], F32, tag="pvv", bufs=2, name="pvv")
                        for kc in range(8):
                            nc.tensor.matmul(
                                pvv[:],
                                vT[:, kc, qc * 128:(qc + 1) * 128],
                                Wvv_sb[:, kc, nch * 512:(nch + 1) * 512],
                                start=(kc == 0),
                                stop=(kc == 7 and not f_bv),
                            )
                        if f_bv:
                            bias_mm_n(
                                pvv[:], bvv_sb, slice(nch * 512, (nch + 1) * 512), 128
                            )
                        vv_ev = p1.tile(
                            [128, 512], F32R, tag="vvev", bufs=3, name="vv_ev"
                        )
                        nc.vector.tensor_copy(vv_ev[:], pvv[:])
                        nc.sync.dma_start(
                            valv_d[:, blk * QCB + qc, nch * 512:(nch + 1) * 512],
                            vv_ev[:],
                        )

        wres1.__exit__(None, None, None)

        # ================= phase 2: attention =================
        with tc.tile_pool(name="ph2", bufs=1) as p2, \
             tc.tile_pool(name="ps2w", bufs=1, space="PSUM") as psW, \
             tc.tile_pool(name="ps2a", bufs=1, space="PSUM") as psA:
            bvo_sb = None
            if f_bvo:
                bvo_sb = p2.tile([1, DV], F32R, name="bvo_sb")
                nc.sync.dma_start(bvo_sb[:], b_v2o[None, :])
            for blk in range(NBLK):
                qT_b = p2.tile([128, 8, BLK], F32R, tag="qTb", bufs=2, name="qT_b")
                nc.sync.dma_start(qT_b[:], qT_d[:, :, blk * BLK:(blk + 1) * BLK])
                vv_b = p2.tile([128, QCB, E], F32R, tag="vvb", bufs=1, name="vv_b")
                nc.sync.dma_start(vv_b[:], valv_d[:, blk * QCB:(blk + 1) * QCB, :])
                r_acc = p2.tile([128, QCB, H], F32, tag="racc", bufs=2, name="r_acc")

                # ---- pass A: scores [q,k], e, out_l accumulation ----
                for hp in range(HP):
                    pol = [
                        psA.tile([64, NL], F32, tag="acc", bufs=4, name=f"pol{j}")
                        for j in range(2)
                    ]
                    for qc in range(QCB):
                        for j in range(2):
                            h = 2 * hp + j
                            ps_s = psW.tile([128, NL], F32, tag="wk", bufs=4, name="ps_s")
                            nc.tensor.matmul(
                                ps_s[:],
                                qT_b[64 * j:64 * (j + 1), hp, qc * 128:(qc + 1) * 128],
                                kT[64 * j:64 * (j + 1), hp, :],
                                start=True,
                                stop=True,
                                tile_position=(64 * j, 0),
                            )
                            e_t = p2.tile([128, NL], F32R, tag="e", bufs=3, name="e_t")
                            nc.scalar.activation(
                                e_t[:], ps_s[:], EXP, scale=SCALE,
                                accum_out=r_acc[:, qc, h:h + 1],
                            )
                            nc.tensor.matmul(
                                pol[j][:],
                                vv_b[:, qc, 64 * h:64 * h + 64],
                                e_t[:],
                                start=(qc == 0),
                                stop=(qc == QCB - 1),
                            )
                    for j in range(2):
                        if blk == 0:
                            nc.vector.tensor_copy(outl_h[j][:, hp, :], pol[j][:])
                        else:
                            nc.vector.tensor_tensor(
                                outl_h[j][:, hp, :], outl_h[j][:, hp, :],
                                pol[j][:], ADD,
                            )

                # ---- r -> reciprocal -> row layout -> divisor tiles ----
                if not gen_mask:
                    rrows = p2.tile([16, BLK], F32, tag="rrows", bufs=2, name="rrows")
                    for qc in range(QCB):
                        r_rec = p2.tile([128, H], F32, tag="rrec", bufs=2, name="r_rec")
                        nc.vector.reciprocal(r_rec[:], r_acc[:, qc, :])
                        pr = psW.tile([16, 128], F32, tag="wk", bufs=4, name="pr")
                        nc.tensor.transpose(pr[:], r_rec[:], ident[:])
                        nc.vector.tensor_copy(
                            rrows[:, qc * 128:(qc + 1) * 128], pr[:]
                        )
                    dram_r = dram.tile([H, BLK], F32, tag="dram_r", bufs=2, name="dram_r")
                    nc.sync.dma_start(dram_r[:], rrows[:])

                # ---- pass B: scores^T, e^T, out_v^T, v2out ----
                outv_h = [
                    p2.tile([64, HP, BLK], F32R, tag=f"outvh{j}", bufs=1,
                            name=f"outv_h{j}")
                    for j in range(2)
                ]
                div64 = None
                for hp in range(HP):
                    if not gen_mask and hp % 4 == 0:
                        # 1/r rows for heads [8*(hp//4), 8*(hp//4)+8), bcast
                        # over 64 partitions; two half-loads to save SBUF
                        div64 = p2.tile(
                            [64, 8, BLK], F32, tag="divall", bufs=1, name="div64"
                        )
                        nc.sync.dma_start(
                            div64[:],
                            dram_r[None, 8 * (hp // 4):8 * (hp // 4) + 8, :]
                            .to_broadcast([64, 8, BLK]),
                        )
                    pov = [
                        psA.tile([64, BLK], F32, tag="acc", bufs=4, name=f"pov{j}")
                        for j in range(2)
                    ]
                    eT_pair = []
                    for j in range(2):
                        h = 2 * hp + j
                        for kc in range(2):
                            ps_t = psW.tile([128, BLK], F32, tag="wk", bufs=4, name="ps_t")
                            nc.tensor.matmul(
                                ps_t[:],
                                kT[64 * j:64 * (j + 1), hp, kc * 128:(kc + 1) * 128],
                                qT_b[64 * j:64 * (j + 1), hp, :],
                                start=True,
                                stop=True,
                                tile_position=(64 * j, 0),
                            )
                            eT_t = p2.tile([128, BLK], F32R, tag="eT", bufs=3, name="eT_t")
                            nc.scalar.activation(
                                eT_t[:], ps_t[:], EXP, scale=SCALE,
                                accum_out=c_acc[:, kc, h, blk:blk + 1],
                            )
                            if gen_mask:
                                nc.vector.tensor_scalar_mul(
                                    eT_t[:], eT_t[:], m01_sb[:, kc:kc + 1]
                                )
                            nc.tensor.matmul(
                                pov[j][:],
                                vall[:, kc, 64 * h:64 * h + 64],
                                eT_t[:],
                                start=(kc == 0),
                                stop=(kc == 1),
                            )
                            eT_pair.append(eT_t)
                    if gen_mask:
                        # masked row-sums via ones-matmul -> recip -> DRAM rows
                        dram_rg = dram.tile(
                            [2, BLK], F32, tag="dram_rg", bufs=3, name="dram_rg"
                        )
                        for j in range(2):
                            prr = psW.tile([1, BLK], F32, tag="wk", bufs=4, name="prr")
                            for kc in range(2):
                                nc.tensor.matmul(
                                    prr[:],
                                    ones128[:],
                                    eT_pair[2 * j + kc][:],
                                    start=(kc == 0),
                                    stop=(kc == 1),
                                )
                            rr_sb = p2.tile([1, BLK], F32, tag="rrsb", bufs=3, name="rr_sb")
                            nc.vector.reciprocal(rr_sb[:], prr[:])
                            nc.sync.dma_start(dram_rg[j:j + 1, :], rr_sb[:])
                        div_g = p2.tile([64, 2, BLK], F32, tag="divhp", bufs=3, name="div_g")
                        nc.sync.dma_start(
                            div_g[:], dram_rg[None, :, :].to_broadcast([64, 2, BLK])
                        )
                        divs = [div_g[:, 0, :], div_g[:, 1, :]]
                    else:
                        divs = [
                            div64[:, (2 * hp) % 8, :],
                            div64[:, (2 * hp + 1) % 8, :],
                        ]
                    for j in range(2):
                        nc.vector.tensor_tensor(
                            outv_h[j][:, hp, :], pov[j][:], divs[j], MULT
                        )
                # repack the two 64-partition halves into [128, 8, BLK]
                outv_sb = p2.tile([128, 8, BLK], F32R, tag="outv", bufs=1, name="outv_sb")
                for j in range(2):
                    nc.sync.dma_start(
                        outv_sb[64 * j:64 * (j + 1), :, :], outv_h[j][:]
                    )

                for qc in range(QCB):
                    for nch in range(2):
                        pf = psW.tile([128, 512], F32, tag="wk", bufs=4, name="pf")
                        for kc in range(8):
                            nc.tensor.matmul(
                                pf[:],
                                outv_sb[:, kc, qc * 128:(qc + 1) * 128],
                                Wvo_sb[:, kc, nch * 512:(nch + 1) * 512],
                                start=(kc == 0),
                                stop=(kc == 7 and not f_bvo),
                            )
                        if f_bvo:
                            bias_mm_n(
                                pf[:], bvo_sb, slice(nch * 512, (nch + 1) * 512), 128
                            )
                        fv = p2.tile([128, 512], F32, tag="fv", bufs=2, name="fv")
                        nc.vector.tensor_copy(fv[:], pf[:])
                        nc.sync.dma_start(
                            out_v[
                                blk * BLK + qc * 128:blk * BLK + (qc + 1) * 128,
                                nch * 512:(nch + 1) * 512,
                            ],
                            fv[:],
                        )

        # ================= phase 3: out_l merge + projection =================
        with tc.tile_pool(name="ph3", bufs=1) as p3, \
             tc.tile_pool(name="ps3", bufs=1, space="PSUM") as ps3:
            c_sum = p3.tile([128, 2, H], F32, name="c_sum")
            nc.vector.tensor_reduce(c_sum[:], c_acc[:], axis=AX, op=ADD)
            outl128 = p3.tile([128, HP, NL], F32, name="outl128")
            for j in range(2):
                nc.sync.dma_start(
                    outl128[64 * j:64 * (j + 1), :, :], outl_h[j][:]
                )
            nc.sync.dma_start(
                ar_in[:, 0:8 * NL], outl128.rearrange("p a b -> p (a b)")
            )
            nc.sync.dma_start(
                ar_in[:, 8 * NL:8 * NL + 2 * H], c_sum.rearrange("p a b -> p (a b)")
            )
            nc.gpsimd.collective_compute(
                "AllReduce",
                ADD,
                replica_groups=RG,
                ins=[ar_in.opt()],
                outs=[ar_out.opt()],
            )
            outl_sum = p3.tile([128, 8, NL], F32, name="outl_sum")
            nc.sync.dma_start(
                outl_sum.rearrange("p a b -> p (a b)"), ar_out[:, 0:8 * NL]
            )
            c_fin = p3.tile([128, 2, H], F32, name="c_fin")
            nc.sync.dma_start(
                c_fin.rearrange("p a b -> p (a b)"), ar_out[:, 8 * NL:8 * NL + 2 * H]
            )
            c_rec = p3.tile([128, 2, H], F32, name="c_rec")
            nc.vector.reciprocal(c_rec[:], c_fin[:])
            crows = p3.tile([16, NL], F32, name="crows")
            for kc in range(2):
                pc = ps3.tile([16, 128], F32, tag="wk3", bufs=2, name="pc")
                nc.tensor.transpose(pc[:], c_rec[:, kc, :], ident[:])
                nc.vector.tensor_copy(crows[:, kc * 128:(kc + 1) * 128], pc[:])
            nc.sync.dma_start(dram_c[:], crows[:])
            divl_all = p3.tile([128, HP, NL], F32, name="divl_all")
            bcast_rows_to_div(divl_all, dram_c, HP, NL)
            outl_n = p3.tile([128, 8, NL], F32R, name="outl_n")
            for hp in range(HP):
                nc.vector.tensor_tensor(
                    outl_n[:, hp, :], outl_sum[:, hp, :], divl_all[:, hp, :], MULT
                )
            Wlo_sb = p3.tile([128, 8, DL], F32R, name="Wlo_sb")
            nc.sync.dma_start(Wlo_sb[:], W_l2o.rearrange("(kc ki) m -> ki kc m", ki=128))
            blo_sb = None
            if f_blo:
                blo_sb = p3.tile([1, DL], F32R, name="blo_sb")
                nc.sync.dma_start(blo_sb[:], b_l2o[None, :])
            for mc2 in range(2):
                for n0, nw in ((0, 512), (512, 256)):
                    pl = ps3.tile([128, 512], F32, tag="wk3", bufs=2, name="pl")
                    for kc in range(8):
                        nc.tensor.matmul(
                            pl[:, 0:nw],
                            outl_n[:, kc, mc2 * 128:(mc2 + 1) * 128],
                            Wlo_sb[:, kc, n0:n0 + nw],
                            start=(kc == 0),
                            stop=(kc == 7 and not f_blo),
                        )
                    if f_blo:
                        bias_mm_n(pl[:, 0:nw], blo_sb, slice(n0, n0 + nw), 128)
                    fl = p3.tile([128, 512], F32, tag="fl", bufs=2, name="fl")
                    nc.vector.tensor_copy(fl[:, 0:nw], pl[:, 0:nw])
                    nc.sync.dma_start(
                        out_l[mc2 * 128:(mc2 + 1) * 128, n0:n0 + nw], fl[:, 0:nw]
                    )

    nc.compile()
    return nc


_cache = {}
_PROFILE = False
LAST_EXEC_NS = None


def _get_nc(flags):
    if flags not in _cache:
        _cache[flags] = _build(flags)
    return _cache[flags]


def kernel(**inputs):
    v = np.asarray(inputs["v"], dtype=np.float32)
    l = np.asarray(inputs["l"], dtype=np.float32)
    mask = np.asarray(inputs["attention_mask_l"])
    ws = {
        k: np.ascontiguousarray(np.asarray(inputs[k], dtype=np.float32))
        for k in ["W_v2q", "W_l2k", "W_v2v", "W_l2v", "W_v2out", "W_l2out"]
    }
    bs = {
        k: np.asarray(inputs[k], dtype=np.float32)
        for k in ["b_v2q", "b_l2k", "b_v2v", "b_l2v", "b_v2out", "b_l2out"]
    }
    flags_b = tuple(
        bool(np.any(bs[k]))
        for k in ["b_v2q", "b_l2k", "b_v2v", "b_l2v", "b_v2out", "b_l2out"]
    )
    gen_mask = bool(np.any(mask == 0))
    flags = flags_b + (gen_mask,)
    nc = _get_nc(flags)

    in_maps = []
    for c in range(N_CORES):
        b, half = c // 2, c % 2
        m = {
            "v": np.ascontiguousarray(v[b, half * NVH:(half + 1) * NVH]),
            "l": np.ascontiguousarray(l[b]),
        }
        m.update(ws)
        for k, f in zip(["b_v2q", "b_l2k", "b_v2v", "b_l2v", "b_v2out", "b_l2out"],
                        flags_b):
            if f:
                m[k] = bs[k]
        if gen_mask:
            mk = mask[b]
            m["mask_bias"] = np.where(mk == 0, -1e9, 0.0).astype(np.float32).reshape(2, 128)
            m["mask01"] = (mk != 0).astype(np.float32).reshape(2, 128)
        in_maps.append(m)

    global LAST_EXEC_NS
    res = run_bass_kernel_spmd(
        nc, in_maps, list(range(N_CORES)), trace=_PROFILE
    )
    LAST_EXEC_NS = res.exec_time_ns
    out_v = np.empty((B, NV, DV), dtype=np.float32)
    out_l = np.empty((B, NL, DL), dtype=np.float32)
    for c in range(N_CORES):
        b, half = c // 2, c % 2
        out_v[b, half * NVH:(half + 1) * NVH] = res.results[c]["out_v"]
        if half == 0:
            out_l[b] = res.results[c]["out_l"]
    return out_v, out_l


# revision 41
# speedup vs baseline: 1.0497x; 1.0175x over previous
"""Bi-directional multi-head cross-attention (GLIP-style) on 8 Trainium2 cores.

Sharding: core c handles (batch = c // 2, v-row half = c % 2) -> 2048 v rows.
 - out_v is fully local to each core (softmax over Nl is within-core).
 - out_l needs reductions over all Nv rows of a batch: each core computes
   unnormalized partial out_l^T = e^T @ val_v and partial colsums c; a 2-rank
   AllReduce per core pair merges them, then both cores finish l2out.

Layouts: activations flow "transposed" (features on partitions) where that
makes every matmul consume natural SBUF layouts with zero runtime transposes
except one PE-transpose of v (and l) at load time.

All matmuls run in float32r (full PE rate, ~1.5e-4 rel err vs fp32).
"""

import contextlib
import sys

sys.path.insert(0, "/opt/trn_rl_repo")

import numpy as np  # noqa: E402

import concourse.bass as bass  # noqa: E402, F401
import concourse.mybir as mybir  # noqa: E402
import concourse.tile as tile  # noqa: E402
from concourse import bacc  # noqa: E402
from concourse.bass_utils import run_bass_kernel_spmd  # noqa: E402
from concourse.masks import make_identity  # noqa: E402

F32 = mybir.dt.float32
F32R = mybir.dt.float32r
EXP = mybir.ActivationFunctionType.Exp
ADD = mybir.AluOpType.add
MULT = mybir.AluOpType.mult
AX = mybir.AxisListType.X

B, NV, NL, DV, DL, E, H, D = 4, 4096, 256, 1024, 768, 1024, 16, 64
NVH = NV // 2  # v rows per core
SCALE = H ** (-0.5)
HP = H // 2  # head pairs
BLK = 512  # q-row block in attention phase
NBLK = NVH // BLK  # 4
QCB = BLK // 128  # q-chunks (128 rows) per block
N_CORES = 8
RG = [[0, 1], [2, 3], [4, 5], [6, 7]]  # core pairs sharing a batch


def _build(flags):
    (f_bq, f_bk, f_bv, f_bl, f_bvo, f_blo, gen_mask) = flags
    nc = bacc.Bacc(
        "TRN2", target_bir_lowering=False, debug=False, num_devices=N_CORES
    )

    v_d = nc.dram_tensor("v", [NVH, DV], F32R, kind="ExternalInput").ap()
    l_d = nc.dram_tensor("l", [NL, DL], F32R, kind="ExternalInput").ap()
    W_v2q = nc.dram_tensor("W_v2q", [DV, E], F32R, kind="ExternalInput").ap()
    W_l2k = nc.dram_tensor("W_l2k", [DL, E], F32R, kind="ExternalInput").ap()
    W_v2v = nc.dram_tensor("W_v2v", [DV, E], F32R, kind="ExternalInput").ap()
    W_l2v = nc.dram_tensor("W_l2v", [DL, E], F32R, kind="ExternalInput").ap()
    W_v2o = nc.dram_tensor("W_v2out", [E, DV], F32R, kind="ExternalInput").ap()
    W_l2o = nc.dram_tensor("W_l2out", [E, DL], F32R, kind="ExternalInput").ap()
    b_v2q = nc.dram_tensor("b_v2q", [E], F32R, kind="ExternalInput").ap() if f_bq else None
    b_l2k = nc.dram_tensor("b_l2k", [E], F32R, kind="ExternalInput").ap() if f_bk else None
    b_v2v = nc.dram_tensor("b_v2v", [E], F32R, kind="ExternalInput").ap() if f_bv else None
    b_l2v = nc.dram_tensor("b_l2v", [E], F32R, kind="ExternalInput").ap() if f_bl else None
    b_v2o = nc.dram_tensor("b_v2out", [DV], F32R, kind="ExternalInput").ap() if f_bvo else None
    b_l2o = nc.dram_tensor("b_l2out", [DL], F32R, kind="ExternalInput").ap() if f_blo else None
    mask_b = (
        nc.dram_tensor("mask_bias", [2, 128], F32, kind="ExternalInput").ap()
        if gen_mask else None
    )
    mask01 = (
        nc.dram_tensor("mask01", [2, 128], F32, kind="ExternalInput").ap()
        if gen_mask else None
    )
    out_v = nc.dram_tensor("out_v", [NVH, DV], F32, kind="ExternalOutput").ap()
    out_l = nc.dram_tensor("out_l", [NL, DL], F32, kind="ExternalOutput").ap()

    any_bias = any([f_bq, f_bk, f_bv, f_bl, f_bvo, f_blo])

    with tile.TileContext(nc) as tc, contextlib.ExitStack() as stack:
        # ---- whole-kernel resident pool ----
        res = stack.enter_context(tc.tile_pool(name="res", bufs=1))
        ident = res.tile([128, 128], F32, name="ident")
        make_identity(nc, ident[:])
        ident_r = res.tile([128, 128], F32R, name="ident_r")
        nc.vector.tensor_copy(ident_r[:], ident[:])
        kT = res.tile([128, 8, NL], F32R, name="kT")  # k^T  [emb, k-rows]
        vall = res.tile([128, 2, E], F32R, name="vall")  # val_l natural [k, emb]
        # out_l^T accum, in per-head 64-partition halves (j = head parity)
        outl_h = [
            res.tile([64, HP, NL], F32, name=f"outl_h{j}") for j in range(2)
        ]
        c_acc = res.tile([128, 2, H, NBLK], F32, name="c_acc")  # colsum partials
        if any_bias:
            ones_f = res.tile([1, 512], F32, name="ones_f")
            nc.vector.memset(ones_f[:], 1.0)
            ones_r = res.tile([1, 512], F32R, name="ones_r")
            nc.vector.tensor_copy(ones_r[:], ones_f[:])
        if gen_mask:
            mb_sb = res.tile([128, 2], F32, name="mb_sb")
            nc.sync.dma_start(mb_sb[:], mask_b.rearrange("kc ki -> ki kc"))
            m01_sb = res.tile([128, 2], F32, name="m01_sb")
            nc.sync.dma_start(m01_sb[:], mask01.rearrange("kc ki -> ki kc"))
            ones128_f = res.tile([128, 1], F32, name="ones128_f")
            nc.vector.memset(ones128_f[:], 1.0)
            ones128 = res.tile([128, 1], F32R, name="ones128")
            nc.vector.tensor_copy(ones128[:], ones128_f[:])

        # v2q/v2v weights: freed after phase 1 (nested pool, closed below)
        wres1 = tc.tile_pool(name="wres1", bufs=1)
        w1 = wres1.__enter__()
        Wq_sb = w1.tile([128, 8, E], F32R, name="Wq_sb")
        nc.sync.dma_start(Wq_sb[:], W_v2q.rearrange("(kc ki) m -> ki kc m", ki=128))
        Wvv_sb = w1.tile([128, 8, E], F32R, name="Wvv_sb")
        nc.sync.dma_start(Wvv_sb[:], W_v2v.rearrange("(kc ki) m -> ki kc m", ki=128))

        dram = stack.enter_context(tc.tile_pool(name="dram", bufs=1, space="DRAM"))
        qT_d = dram.tile([128, 8, NVH], F32R, name="qT_d")
        valv_d = dram.tile([128, NVH // 128, E], F32R, name="valv_d")
        ar_in = dram.tile([128, 8 * NL + 2 * H], F32, name="ar_in")
        ar_out = dram.tile([128, 8 * NL + 2 * H], F32, name="ar_out")
        dram_c = dram.tile([H, NL], F32, name="dram_c")

        def bcast_rows_to_div(div_dst, rows_src, hp_count, n):
            # div_dst [128, hp_count, n] <- rows_src [2*hp_count, n] (DRAM),
            # partition p of slice hp reads row 2*hp + (p >= 64).
            src = rows_src.rearrange("(hp j) q -> j hp q", j=2)
            for j in range(2):
                nc.sync.dma_start(
                    div_dst[64 * j:64 * (j + 1)],
                    src[j, None, :, :].to_broadcast([64, hp_count, n]),
                )

        def bias_mm_t(ps, b_sb, mslice, nsize):
            # transposed-layout output: out[emb, rows] += b[emb]
            nc.tensor.matmul(
                ps, b_sb[0:1, mslice], ones_r[0:1, 0:nsize], start=False, stop=True
            )

        def bias_mm_n(ps, b_sb, nslice, msize):
            # natural-layout output: out[rows, emb] += b[emb]
            nc.tensor.matmul(
                ps, ones_r[0:1, 0:msize], b_sb[0:1, nslice], start=False, stop=True
            )

        # ===== phases 0+1 share one pool scope so their DMAs/PE work can
        # interleave (separate pools would serialize on address reuse) =====
        with tc.tile_pool(name="ph0", bufs=1) as p0, \
             tc.tile_pool(name="ph1", bufs=1) as p1, \
             tc.tile_pool(name="ps0", bufs=1, space="PSUM") as ps0:
            ps1 = ps0
            # ---- phase 0: l-side projections ----
            l_nat = p0.tile([128, 2, DL], F32R, name="l_nat")
            nc.sync.dma_start(l_nat[:], l_d.rearrange("(ko ki) f -> ki ko f", ki=128))
            lT = p0.tile([128, 6, NL], F32R, name="lT")
            for fc in range(6):
                for lc in range(2):
                    pt = ps0.tile([128, 128], F32R, tag="tr", bufs=3, name="pt")
                    nc.tensor.transpose(
                        pt[:], l_nat[:, lc, fc * 128:(fc + 1) * 128], ident_r[:]
                    )
                    nc.vector.tensor_copy(lT[:, fc, lc * 128:(lc + 1) * 128], pt[:])
            Wk_sb = p0.tile([128, 6, E], F32R, tag="wl", bufs=1, name="Wk_sb")
            nc.sync.dma_start(Wk_sb[:], W_l2k.rearrange("(kc ki) m -> ki kc m", ki=128))
            bk_sb = None
            if f_bk:
                bk_sb = p0.tile([1, E], F32R, name="bk_sb")
                nc.sync.dma_start(bk_sb[:], b_l2k[None, :])
            blv_sb = None
            if f_bl:
                blv_sb = p0.tile([1, E], F32R, name="blv_sb")
                nc.sync.dma_start(blv_sb[:], b_l2v[None, :])
            for mc in range(8):
                pk = ps0.tile([128, 512], F32, tag="pp", bufs=2, name="pk")[:, 0:NL]
                for kc in range(6):
                    nc.tensor.matmul(
                        pk[:],
                        Wk_sb[:, kc, mc * 128:(mc + 1) * 128],
                        lT[:, kc, :],
                        start=(kc == 0),
                        stop=(kc == 5 and not f_bk),
                    )
                if f_bk:
                    bias_mm_t(pk[:], bk_sb, slice(mc * 128, (mc + 1) * 128), NL)
                nc.vector.tensor_copy(kT[:, mc, :], pk[:])
            Wlv_sb = p0.tile([128, 6, E], F32R, tag="wl", bufs=1, name="Wlv_sb")
            nc.sync.dma_start(Wlv_sb[:], W_l2v.rearrange("(kc ki) m -> ki kc m", ki=128))
            for mc2 in range(2):
                for nch in range(2):
                    pv = ps0.tile([128, 512], F32, tag="pw", bufs=2, name="pv")
                    for kc in range(6):
                        nc.tensor.matmul(
                            pv[:],
                            lT[:, kc, mc2 * 128:(mc2 + 1) * 128],
                            Wlv_sb[:, kc, nch * 512:(nch + 1) * 512],
                            start=(kc == 0),
                            stop=(kc == 5 and not f_bl),
                        )
                    if f_bl:
                        bias_mm_n(pv[:], blv_sb, slice(nch * 512, (nch + 1) * 512), 128)
                    nc.vector.tensor_copy(
                        vall[:, mc2, nch * 512:(nch + 1) * 512], pv[:]
                    )

            # ---- phase 1: v-side projections ----
            bq_sb = None
            if f_bq:
                bq_sb = p1.tile([1, E], F32R, name="bq_sb")
                nc.sync.dma_start(bq_sb[:], b_v2q[None, :])
            bvv_sb = None
            if f_bv:
                bvv_sb = p1.tile([1, E], F32R, name="bvv_sb")
                nc.sync.dma_start(bvv_sb[:], b_v2v[None, :])
            for blk in range(NBLK):
                v_nat = p1.tile([128, QCB, DV], F32R, tag="vnat", bufs=2, name="v_nat")
                nc.sync.dma_start(
                    v_nat[:],
                    v_d[blk * BLK:(blk + 1) * BLK, :].rearrange(
                        "(qo qi) f -> qi qo f", qi=128
                    ),
                )
                vT = p1.tile([128, 8, BLK], F32R, tag="vT", bufs=2, name="vT")
                for fc in range(8):
                    for qc in range(QCB):
                        pt = ps1.tile([128, 128], F32R, tag="tr", bufs=3, name="pt1")
                        nc.tensor.transpose(
                            pt[:], v_nat[:, qc, fc * 128:(fc + 1) * 128], ident_r[:]
                        )
                        nc.vector.tensor_copy(
                            vT[:, fc, qc * 128:(qc + 1) * 128], pt[:]
                        )
                for mc in range(8):
                    pq = ps1.tile([128, BLK], F32, tag="pp", bufs=2, name="pq")
                    for kc in range(8):
                        nc.tensor.matmul(
                            pq[:],
                            Wq_sb[:, kc, mc * 128:(mc + 1) * 128],
                            vT[:, kc, :],
                            start=(kc == 0),
                            stop=(kc == 7 and not f_bq),
                        )
                    if f_bq:
                        bias_mm_t(pq[:], bq_sb, slice(mc * 128, (mc + 1) * 128), BLK)
                    qt_ev = p1.tile([128, BLK], F32R, tag="qtev", bufs=2, name="qt_ev")
                    nc.vector.tensor_copy(qt_ev[:], pq[:])
                    nc.sync.dma_start(
                        qT_d[:, mc, blk * BLK:(blk + 1) * BLK], qt_ev[:]
                    )
                for qc in range(QCB):
                    for nch in range(2):
                        pvv = ps1.tile([128, 512], F32, tag="pw", bufs=2, name="pvv")
                        for kc in range(8):
                            nc.tensor.matmul(
                                pvv[:],
                                vT[:, kc, qc * 128:(qc + 1) * 128],
                                Wvv_sb[:, kc, nch * 512:(nch + 1) * 512],
                                start=(kc == 0),
                                stop=(kc == 7 and not f_bv),
                            )
                        if f_bv:
                            bias_mm_n(
                                pvv[:], bvv_sb, slice(nch * 512, (nch + 1) * 512), 128
                            )
                        vv_ev = p1.tile(
                            [128, 512], F32R, tag="vvev", bufs=2, name="vv_ev"
                        )
                        nc.vector.tensor_copy(vv_ev[:], pvv[:])
                        nc.sync.dma_start(
                            valv_d[:, blk * QCB + qc, nch * 512:(nch + 1) * 512],
                            vv_ev[:],
                        )

        wres1.__exit__(None, None, None)

        # ================= phase 2: attention =================
        with tc.tile_pool(name="ph2", bufs=1) as p2, \
             tc.tile_pool(name="ps2w", bufs=1, space="PSUM") as psW, \
             tc.tile_pool(name="ps2a", bufs=1, space="PSUM") as psA:
            Wvo_sb = p2.tile([128, 8, DV], F32R, name="Wvo_sb")
            nc.sync.dma_start(Wvo_sb[:], W_v2o.rearrange("(kc ki) m -> ki kc m", ki=128))
            bvo_sb = None
            if f_bvo:
                bvo_sb = p2.tile([1, DV], F32R, name="bvo_sb")
                nc.sync.dma_start(bvo_sb[:], b_v2o[None, :])
            for blk in range(NBLK):
                qT_b = p2.tile([128, 8, BLK], F32R, tag="qTb", bufs=2, name="qT_b")
                nc.sync.dma_start(qT_b[:], qT_d[:, :, blk * BLK:(blk + 1) * BLK])
                vv_b = p2.tile([128, QCB, E], F32R, tag="vvb", bufs=1, name="vv_b")
                nc.sync.dma_start(vv_b[:], valv_d[:, blk * QCB:(blk + 1) * QCB, :])
                r_acc = p2.tile([128, QCB, H], F32, tag="racc", bufs=2, name="r_acc")

                # ---- pass A: scores [q,k], e, out_l accumulation ----
                for hp in range(HP):
                    pol = [
                        psA.tile([64, NL], F32, tag="acc", bufs=4, name=f"pol{j}")
                        for j in range(2)
                    ]
                    for qc in range(QCB):
                        for j in range(2):
                            h = 2 * hp + j
                            ps_s = psW.tile([128, NL], F32, tag="wk", bufs=4, name="ps_s")
                            nc.tensor.matmul(
                                ps_s[:],
                                qT_b[64 * j:64 * (j + 1), hp, qc * 128:(qc + 1) * 128],
                                kT[64 * j:64 * (j + 1), hp, :],
                                start=True,
                                stop=True,
                                tile_position=(64 * j, 0),
                            )
                            e_t = p2.tile([128, NL], F32R, tag="e", bufs=6, name="e_t")
                            nc.scalar.activation(
                                e_t[:], ps_s[:], EXP, scale=SCALE,
                                accum_out=r_acc[:, qc, h:h + 1],
                            )
                            nc.tensor.matmul(
                                pol[j][:],
                                vv_b[:, qc, 64 * h:64 * h + 64],
                                e_t[:],
                                start=(qc == 0),
                                stop=(qc == QCB - 1),
                            )
                    for j in range(2):
                        if blk == 0:
                            nc.vector.tensor_copy(outl_h[j][:, hp, :], pol[j][:])
                        else:
                            nc.vector.tensor_tensor(
                                outl_h[j][:, hp, :], outl_h[j][:, hp, :],
                                pol[j][:], ADD,
                            )

                # ---- r -> reciprocal -> row layout -> divisor tiles ----
                if not gen_mask:
                    rrows = p2.tile([16, BLK], F32, tag="rrows", bufs=2, name="rrows")
                    for qc in range(QCB):
                        r_rec = p2.tile([128, H], F32, tag="rrec", bufs=2, name="r_rec")
                        nc.vector.reciprocal(r_rec[:], r_acc[:, qc, :])
                        pr = psW.tile([16, 128], F32, tag="wk", bufs=4, name="pr")
                        nc.tensor.transpose(pr[:], r_rec[:], ident[:])
                        nc.vector.tensor_copy(
                            rrows[:, qc * 128:(qc + 1) * 128], pr[:]
                        )
                    dram_r = dram.tile([H, BLK], F32, tag="dram_r", bufs=2, name="dram_r")
                    nc.sync.dma_start(dram_r[:], rrows[:])

                # ---- pass B: scores^T, e^T, out_v^T, v2out ----
                outv_h = [
                    p2.tile([64, HP, BLK], F32R, tag=f"outvh{j}", bufs=1,
                            name=f"outv_h{j}")
                    for j in range(2)
                ]
                div64 = None
                for hp in range(HP):
                    if not gen_mask and hp % 4 == 0:
                        # 1/r rows for heads [8*(hp//4), 8*(hp//4)+8), bcast
                        # over 64 partitions; two half-loads to save SBUF
                        div64 = p2.tile(
                            [64, 8, BLK], F32, tag="divall", bufs=1, name="div64"
                        )
                        nc.sync.dma_start(
                            div64[:],
                            dram_r[None, 8 * (hp // 4):8 * (hp // 4) + 8, :]
                            .to_broadcast([64, 8, BLK]),
                        )
                    pov = [
                        psA.tile([64, BLK], F32, tag="acc", bufs=4, name=f"pov{j}")
                        for j in range(2)
                    ]
                    eT_pair = []
                    for j in range(2):
                        h = 2 * hp + j
                        for kc in range(2):
                            ps_t = psW.tile([128, BLK], F32, tag="wk", bufs=4, name="ps_t")
                            nc.tensor.matmul(
                                ps_t[:],
                                kT[64 * j:64 * (j + 1), hp, kc * 128:(kc + 1) * 128],
                                qT_b[64 * j:64 * (j + 1), hp, :],
                                start=True,
                                stop=True,
                                tile_position=(64 * j, 0),
                            )
                            eT_t = p2.tile([128, BLK], F32R, tag="eT", bufs=4, name="eT_t")
                            nc.scalar.activation(
                                eT_t[:], ps_t[:], EXP, scale=SCALE,
                                accum_out=c_acc[:, kc, h, blk:blk + 1],
                            )
                            if gen_mask:
                                nc.vector.tensor_scalar_mul(
                                    eT_t[:], eT_t[:], m01_sb[:, kc:kc + 1]
                                )
                            nc.tensor.matmul(
                                pov[j][:],
                                vall[:, kc, 64 * h:64 * h + 64],
                                eT_t[:],
                                start=(kc == 0),
                                stop=(kc == 1),
                            )
                            eT_pair.append(eT_t)
                    if gen_mask:
                        # masked row-sums via ones-matmul -> recip -> DRAM rows
                        dram_rg = dram.tile(
                            [2, BLK], F32, tag="dram_rg", bufs=3, name="dram_rg"
                        )
                        for j in range(2):
                            prr = psW.tile([1, BLK], F32, tag="wk", bufs=4, name="prr")
                            for kc in range(2):
                                nc.tensor.matmul(
                                    prr[:],
                                    ones128[:],
                                    eT_pair[2 * j + kc][:],
                                    start=(kc == 0),
                                    stop=(kc == 1),
                                )
                            rr_sb = p2.tile([1, BLK], F32, tag="rrsb", bufs=3, name="rr_sb")
                            nc.vector.reciprocal(rr_sb[:], prr[:])
                            nc.sync.dma_start(dram_rg[j:j + 1, :], rr_sb[:])
                        div_g = p2.tile([64, 2, BLK], F32, tag="divhp", bufs=3, name="div_g")
                        nc.sync.dma_start(
                            div_g[:], dram_rg[None, :, :].to_broadcast([64, 2, BLK])
                        )
                        divs = [div_g[:, 0, :], div_g[:, 1, :]]
                    else:
                        divs = [
                            div64[:, (2 * hp) % 8, :],
                            div64[:, (2 * hp + 1) % 8, :],
                        ]
                    for j in range(2):
                        nc.vector.tensor_tensor(
                            outv_h[j][:, hp, :], pov[j][:], divs[j], MULT
                        )
                # repack the two 64-partition halves into [128, 8, BLK]
                outv_sb = p2.tile([128, 8, BLK], F32R, tag="outv", bufs=1, name="outv_sb")
                for j in range(2):
                    nc.sync.dma_start(
                        outv_sb[64 * j:64 * (j + 1), :, :], outv_h[j][:]
                    )

                for qc in range(QCB):
                    for nch in range(2):
                        pf = psW.tile([128, 512], F32, tag="wk", bufs=4, name="pf")
                        for kc in range(8):
                            nc.tensor.matmul(
                                pf[:],
                                outv_sb[:, kc, qc * 128:(qc + 1) * 128],
                                Wvo_sb[:, kc, nch * 512:(nch + 1) * 512],
                                start=(kc == 0),
                                stop=(kc == 7 and not f_bvo),
                            )
                        if f_bvo:
                            bias_mm_n(
                                pf[:], bvo_sb, slice(nch * 512, (nch + 1) * 512), 128
                            )
                        fv = p2.tile([128, 512], F32, tag="fv", bufs=2, name="fv")
                        nc.vector.tensor_copy(fv[:], pf[:])
                        nc.sync.dma_start(
                            out_v[
                                blk * BLK + qc * 128:blk * BLK + (qc + 1) * 128,
                                nch * 512:(nch + 1) * 512,
                            ],
                            fv[:],
                        )

        # ================= phase 3: out_l merge + projection =================
        with tc.tile_pool(name="ph3", bufs=1) as p3, \
             tc.tile_pool(name="ps3", bufs=1, space="PSUM") as ps3:
            c_sum = p3.tile([128, 2, H], F32, name="c_sum")
            nc.vector.tensor_reduce(c_sum[:], c_acc[:], axis=AX, op=ADD)
            outl128 = p3.tile([128, HP, NL], F32, name="outl128")
            for j in range(2):
                nc.sync.dma_start(
                    outl128[64 * j:64 * (j + 1), :, :], outl_h[j][:]
                )
            nc.sync.dma_start(
                ar_in[:, 0:8 * NL], outl128.rearrange("p a b -> p (a b)")
            )
            nc.sync.dma_start(
                ar_in[:, 8 * NL:8 * NL + 2 * H], c_sum.rearrange("p a b -> p (a b)")
            )
            nc.gpsimd.collective_compute(
                "AllReduce",
                ADD,
                replica_groups=RG,
                ins=[ar_in.opt()],
                outs=[ar_out.opt()],
            )
            outl_sum = p3.tile([128, 8, NL], F32, name="outl_sum")
            nc.sync.dma_start(
                outl_sum.rearrange("p a b -> p (a b)"), ar_out[:, 0:8 * NL]
            )
            c_fin = p3.tile([128, 2, H], F32, name="c_fin")
            nc.sync.dma_start(
                c_fin.rearrange("p a b -> p (a b)"), ar_out[:, 8 * NL:8 * NL + 2 * H]
            )
            c_rec = p3.tile([128, 2, H], F32, name="c_rec")
            nc.vector.reciprocal(c_rec[:], c_fin[:])
            crows = p3.tile([16, NL], F32, name="crows")
            for kc in range(2):
                pc = ps3.tile([16, 128], F32, tag="wk3", bufs=2, name="pc")
                nc.tensor.transpose(pc[:], c_rec[:, kc, :], ident[:])
                nc.vector.tensor_copy(crows[:, kc * 128:(kc + 1) * 128], pc[:])
            nc.sync.dma_start(dram_c[:], crows[:])
            divl_all = p3.tile([128, HP, NL], F32, name="divl_all")
            bcast_rows_to_div(divl_all, dram_c, HP, NL)
            outl_n = p3.tile([128, 8, NL], F32R, name="outl_n")
            for hp in range(HP):
                nc.vector.tensor_tensor(
                    outl_n[:, hp, :], outl_sum[:, hp, :], divl_all[:, hp, :], MULT
                )
            Wlo_sb = p3.tile([128, 8, DL], F32R, name="Wlo_sb")
            nc.sync.dma_start(Wlo_sb[:], W_l2o.rearrange("(kc ki) m -> ki kc m", ki=128))
            blo_sb = None
            if f_blo:
                blo_sb = p3.tile([1, DL], F32R, name="blo_sb")
                nc.sync.dma_start(blo_sb[:], b_l2o[None, :])
            for mc2 in range(2):
                for n0, nw in ((0, 512), (512, 256)):
                    pl = ps3.tile([128, 512], F32, tag="wk3", bufs=2, name="pl")
                    for kc in range(8):
                        nc.tensor.matmul(
                            pl[:, 0:nw],
                            outl_n[:, kc, mc2 * 128:(mc2 + 1) * 128],
                            Wlo_sb[:, kc, n0:n0 + nw],
                            start=(kc == 0),
                            stop=(kc == 7 and not f_blo),
                        )
                    if f_blo:
                        bias_mm_n(pl[:, 0:nw], blo_sb, slice(n0, n0 + nw), 128)
                    fl = p3.tile([128, 512], F32, tag="fl", bufs=2, name="fl")
                    nc.vector.tensor_copy(fl[:, 0:nw], pl[:, 0:nw])
                    nc.sync.dma_start(
                        out_l[mc2 * 128:(mc2 + 1) * 128, n0:n0 + nw], fl[:, 0:nw]
                    )

    nc.compile()
    return nc


_cache = {}
_PROFILE = False
LAST_EXEC_NS = None


def _get_nc(flags):
    if flags not in _cache:
        _cache[flags] = _build(flags)
    return _cache[flags]


def kernel(**inputs):
    v = np.asarray(inputs["v"], dtype=np.float32)
    l = np.asarray(inputs["l"], dtype=np.float32)
    mask = np.asarray(inputs["attention_mask_l"])
    ws = {
        k: np.ascontiguousarray(np.asarray(inputs[k], dtype=np.float32))
        for k in ["W_v2q", "W_l2k", "W_v2v", "W_l2v", "W_v2out", "W_l2out"]
    }
    bs = {
        k: np.asarray(inputs[k], dtype=np.float32)
        for k in ["b_v2q", "b_l2k", "b_v2v", "b_l2v", "b_v2out", "b_l2out"]
    }
    flags_b = tuple(
        bool(np.any(bs[k]))
        for k in ["b_v2q", "b_l2k", "b_v2v", "b_l2v", "b_v2out", "b_l2out"]
    )
    gen_mask = bool(np.any(mask == 0))
    flags = flags_b + (gen_mask,)
    nc = _get_nc(flags)

    in_maps = []
    for c in range(N_CORES):
        b, half = c // 2, c % 2
        m = {
            "v": np.ascontiguousarray(v[b, half * NVH:(half + 1) * NVH]),
            "l": np.ascontiguousarray(l[b]),
        }
        m.update(ws)
        for k, f in zip(["b_v2q", "b_l2k", "b_v2v", "b_l2v", "b_v2out", "b_l2out"],
                        flags_b):
            if f:
                m[k] = bs[k]
        if gen_mask:
            mk = mask[b]
            m["mask_bias"] = np.where(mk == 0, -1e9, 0.0).astype(np.float32).reshape(2, 128)
            m["mask01"] = (mk != 0).astype(np.float32).reshape(2, 128)
        in_maps.append(m)

    global LAST_EXEC_NS
    res = run_bass_kernel_spmd(
        nc, in_maps, list(range(N_CORES)), trace=_PROFILE
    )
    LAST_EXEC_NS = res.exec_time_ns
    out_v = np.empty((B, NV, DV), dtype=np.float32)
    out_l = np.empty((B, NL, DL), dtype=np.float32)
    for c in range(N_CORES):
        b, half = c // 2, c % 2
        out_v[b, half * NVH:(half + 1) * NVH] = res.results[c]["out_v"]
        if half == 0:
            out_l[b] = res.results[c]["out_l"]
    return out_v, out_l
